# revision 20
# baseline (speedup 1.0000x reference)
"""Multi-head attention (b=2, t=2048, d=1024, h=16, hd=64) on 8 trn2 NeuronCores.

Sharding: core c = 4*b + g handles batch b and head-group g (4 heads =
2 fb-pairs, feature columns [g*256, (g+1)*256)). QKV column-sharded,
Wo row-sharded (Megatron); each core emits ONE pair-combined partial
[2048, 1024] f16 output; host sums the 4 group partials per batch and
adds bo_eff = bo + bv @ Wo (the V bias commutes through softmax:
softmax(S) @ (V + 1 bv^T) = softmax(S) @ V + 1 bv^T).

Schedule: 4 rounds (fb-pair, query-half) x 16 key blocks, paced by the
ACT engine (exp: 128 x [128,1024] instrs ~1.34us each, ~171us total);
PE work hides under it:
 - QK: two K=64 matmuls concurrent via row tiles (0,0)/(64,0).
 - PV: col tiles (0,0)/(0,64); head-even ctx to psum rows 0-63,
   head-odd to 64-127.
 - softmax denominators: four M=1 col-tiled matmuls (strips
   0/32/64/96) with a ones stationary vector, one psum bank.
 - 1/denominator: nc.vector.reciprocal_approx_fast.
 - V projected token-major (lhsT = xT chunk): no transposes.
Inputs arrive as 7 batched multi-dim-AP DMAs; the fb0 K/Q projections
run dc-outer, chasing the xT chunk DMAs, into borrowed attention psum
banks. PSUM (8 banks): st ping/pong 4 + ct 2 + den/rep 1 + scratch 1.
"""

import numpy as np

import concourse.bass as bass
import concourse.mybir as mybir
import concourse.tile as tile
from concourse.bass_utils import run_bass_kernel_spmd

F32 = mybir.dt.float32
F16 = mybir.dt.float16
EXP = mybir.ActivationFunctionType.Exp

T = 2048          # tokens per batch
D = 1024          # model dim
HD = 64           # head dim
GF = 256          # features per head-group (4 heads)
NT = T // 128     # 16 token/key blocks

MAX_WAITS = 1


def _split_waits(nc):
    """walrus in this container allows only one sync-wait per instruction;
    hoist extras onto same-engine NoOps immediately before the offender."""
    for f in nc.m.functions:
        for blk in f.blocks:
            insts = list(blk.instructions)
            new, changed = [], False
            for ins in insts:
                si = ins.sync_info
                waits = list(si.on_wait) if si and si.on_wait else []
                if len(waits) > MAX_WAITS:
                    changed = True
                    extra, keep = waits[:-MAX_WAITS], waits[-MAX_WAITS:]
                    for i in range(0, len(extra), MAX_WAITS):
                        new.append(mybir.InstNoOp(
                            name=f"{ins.name}-wsplit{i}",
                            engine=ins.engine,
                            sync_info=mybir.SyncInfo(
                                on_wait=extra[i:i + MAX_WAITS], on_update=[]),
                        ))
                    ins.sync_info = mybir.SyncInfo(
                        on_wait=keep,
                        on_update=list(si.on_update) if si.on_update else [])
                new.append(ins)
            if changed:
                blk.instructions = new


def _build_program():
    nc = bass.Bass("TRN2", target_bir_lowering=False, debug=False, num_devices=8)

    # all inputs host-swizzled to [128, free] SBUF layout, chunk-major
    xT = nc.dram_tensor("xT", [128, 8 * T], F16, kind="ExternalInput")
    Wq = nc.dram_tensor("Wq", [128, 8 * GF], F16, kind="ExternalInput")
    Wk = nc.dram_tensor("Wk", [128, 8 * GF], F16, kind="ExternalInput")
    Wv = nc.dram_tensor("Wv", [128, 8 * GF], F16, kind="ExternalInput")
    Wo = nc.dram_tensor("Wo", [128, 2 * D], F16, kind="ExternalInput")
    bq = nc.dram_tensor("bq", [128, 2], F32, kind="ExternalInput")
    bk = nc.dram_tensor("bk", [128, 2], F32, kind="ExternalInput")
    out = nc.dram_tensor("out", [T, D], F16, kind="ExternalOutput")

    with tile.TileContext(nc) as tc:
        with (
            nc.allow_low_precision(reason="fp16 rounding is intentional"),
            tc.tile_pool(name="w", bufs=1) as wp,        # persistent tiles
            tc.tile_pool(name="pt", bufs=8) as ptp,      # probs tiles
            tc.tile_pool(name="rs", bufs=2) as rsp,      # recip staging
            tc.tile_pool(name="ob", bufs=3) as obp,      # out staging
            tc.tile_pool(name="st", bufs=2, space="PSUM") as pst,   # scores
            tc.tile_pool(name="ct", bufs=1, space="PSUM") as psc,   # ctx accum
            tc.tile_pool(name="dn", bufs=1, space="PSUM") as psd,   # denom/rep
            tc.tile_pool(name="sp", bufs=1, space="PSUM") as psp,   # scratch
        ):
            # ---- ACT table preload: dummy exp at t0 (hides the ~2.7us
            # PSEUDO_LOAD_ACT_FUNC_SET under the input DMAs) ----------------
            warm_i = wp.tile([1, 16], F32, tag="warm_i")
            nc.gpsimd.memset(warm_i[:], 0.0)
            warm_o = wp.tile([1, 16], F16, tag="warm_o")
            nc.scalar.activation(warm_o[:], warm_i[:], EXP, scale=1.0)

            # ---- batched input DMAs. All tensors are pre-swizzled on the
            # host to the exact [128, free] SBUF layout (chunk-major along
            # free), so every DMA is a contiguous per-partition run --------
            wk_all = wp.tile([128, 8 * GF], F16, tag="wk")
            nc.sync.dma_start(wk_all[:], Wk[:])
            wq_all = wp.tile([128, 8 * GF], F16, tag="wq")
            nc.sync.dma_start(wq_all[:], Wq[:])
            xTt = wp.tile([128, 8 * T], F16, tag="xt")
            for lo, hi in ((0, 1), (1, 2), (2, 4), (4, 6), (6, 8)):
                nc.sync.dma_start(
                    xTt[:, lo * T:hi * T], xT[:, lo * T:hi * T])
            wv_all = wp.tile([128, 8 * GF], F16, tag="wv")
            nc.sync.dma_start(wv_all[:], Wv[:])
            bk_all = wp.tile([128, 2], F32, tag="bk")
            nc.sync.dma_start(bk_all[:], bk[:])
            bq_all = wp.tile([128, 2], F32, tag="bq")
            nc.sync.dma_start(bq_all[:], bq[:])
            wo_all = wp.tile([128, 2 * D], F16, tag="wo")
            nc.sync.dma_start(wo_all[:], Wo[:])

            def xc(dc, lo, hi):          # xT chunk dc, token slice
                return xTt[:, dc * T + lo:dc * T + hi]

            def wsl(w, dc, lo, hi):      # weight chunk dc, feature slice
                return w[:, dc * GF + lo:dc * GF + hi]

            onesP = wp.tile([128, 1], F16, tag="onesP")   # denom lhsT
            nc.gpsimd.memset(onesP[:], 1.0)
            onesR = wp.tile([128, 64], F16, tag="onesR")  # recip-replicate lhsT
            nc.gpsimd.memset(onesR[:], 1.0)
            CTn0g = wp.tile([128, 512], F16, tag="wrg")   # warmup rhs (garbage)
            nc.gpsimd.memset(CTn0g[:], 0.0)

            # PE warmup: dummy matmuls (no DMA deps) spanning until the xT
            # chunks land, so the HAM clock-gate is 8/8 for the chains
            wrm = pst.tile([128, 1024], F32, tag="st", name="wrm")
            for i in range(32):
                nc.tensor.matmul(wrm[0:64, 0:512], onesR[:, 0:64],
                                 CTn0g[:, 0:512], start=True, stop=True)

            # Q^T/K^T feature-major: rows 0-63 head-even(2fb), 64-127 head-odd
            QT = [wp.tile([128, T], F16, tag=f"qt{fb}", name=f"qt{fb}")
                  for fb in range(2)]
            KT = [wp.tile([128, T], F16, tag=f"kt{fb}", name=f"kt{fb}")
                  for fb in range(2)]
            # V token-major: [128 tokens, 4 heads x 64]
            V_t = [wp.tile([128, GF], F16, tag=f"v{tb}", name=f"v{tb}")
                   for tb in range(NT)]
            # normalized ctx, pair-feature-major: [128 feats, T]
            CTn = [wp.tile([128, T], F16, tag=f"ctn{fb}", name=f"ctn{fb}")
                   for fb in range(2)]

            # ---- pre-phase: fb0 K/Q projections, dc-outer, chasing the xT
            # chunk DMAs; psum = the 4 attention banks (idle until round 0)
            preK = [pst.tile([128, 1024], F32, tag="st", name=f"preK{i}")
                    for i in range(2)]
            preQ01 = psc.tile([128, 1024], F32, tag="ct", name="preQ01")
            preQ2 = psd.tile([128, 512], F32, tag="dn", name="preQ2")
            preQ3 = psp.tile([128, 512], F32, tag="sp", name="preQ3")
            def pre_mm(kind, tck, dc):
                first, last = dc == 0, dc == 7
                if kind == 0:
                    dst = preK[tck // 2][:,
                                         (tck % 2) * 512:(tck % 2) * 512 + 512]
                    w = wk_all
                else:
                    dst = (preQ01[:, 0:512], preQ01[:, 512:1024],
                           preQ2[:], preQ3[:])[tck]
                    w = wq_all
                nc.tensor.matmul(
                    dst, wsl(w, dc, 0, 128),
                    xc(dc, tck * 512, (tck + 1) * 512),
                    start=first, stop=last)

            for dc in range(7):
                for tck in range(4):
                    pre_mm(0, tck, dc)
                for tck in range(4):
                    pre_mm(1, tck, dc)
            # dc7 in j0-gating order: KT tck0, QT tck0/1 complete first
            for kind, tck in ((0, 0), (1, 0), (1, 1), (0, 1),
                              (0, 2), (0, 3), (1, 2), (1, 3)):
                pre_mm(kind, tck, 7)
            # bias casts, in first-use order (QK j0 needs KT tck0 + QT tck0/1)
            for (dst, src, b_all) in (
                    (KT[0][:, 0:512], preK[0][:, 0:512], bk_all),
                    (QT[0][:, 0:512], preQ01[:, 0:512], bq_all),
                    (QT[0][:, 512:1024], preQ01[:, 512:1024], bq_all),
                    (KT[0][:, 512:1024], preK[0][:, 512:1024], bk_all),
                    (KT[0][:, 1024:1536], preK[1][:, 0:512], bk_all),
                    (KT[0][:, 1536:2048], preK[1][:, 512:1024], bk_all),
                    (QT[0][:, 1024:1536], preQ2[:], bq_all),
                    (QT[0][:, 1536:2048], preQ3[:], bq_all)):
                nc.vector.tensor_scalar_add(dst, src, b_all[:, 0:1])

            # Q/K projection filler halves (fb1), psum via the scratch bank
            def qk_half(w_all, b_all, dst, tck, phase, sp):
                for dc in range(4 * phase, 4 * phase + 4):
                    nc.tensor.matmul(
                        sp[:, 0:512], wsl(w_all, dc, 128, 256),
                        xc(dc, tck * 512, (tck + 1) * 512),
                        start=(dc == 0), stop=(dc == 7))
                if phase == 1:
                    nc.vector.tensor_scalar_add(
                        dst[1][:, tck * 512:(tck + 1) * 512], sp[:, 0:512],
                        b_all[:, 1:2])

            def mk_qkhalf(w_all, b_all, dst, tck):
                st = {}

                def f(phase):
                    if phase == 0:
                        st["sp"] = psp.tile([128, 512], F32, tag="sp",
                                            name="fill_sp")
                    qk_half(w_all, b_all, dst, tck, phase, st["sp"])
                return f

            # V projection, token-major (no bias - folded into host bo_eff)
            def v_unit(tb):
                vp = psp.tile([128, GF], F32, tag="sp", name="vp")
                for dc in range(8):
                    nc.tensor.matmul(
                        vp[:], xc(dc, tb * 128, (tb + 1) * 128),
                        wsl(wv_all, dc, 0, 256),
                        start=(dc == 0), stop=(dc == 7))
                nc.vector.tensor_copy(V_t[tb][:], vp[:])

            # pair-combined output projection for token block tb
            def out_unit(tb, pool, tag, wide):
                ob = obp.tile([128, D], F16, tag="o", name="o")
                spw = pool.tile([128, 1024], F32, tag=tag, name="osp") \
                    if wide else None
                for nck in range(2):
                    if wide:
                        p = spw[:, nck * 512:(nck + 1) * 512]
                    else:
                        sp = pool.tile([128, 512], F32, tag=tag, name="osp")
                        p = sp[:, 0:512]
                    nc.tensor.matmul(
                        p, CTn[0][:, tb * 128:(tb + 1) * 128],
                        wo_all[:, nck * 512:(nck + 1) * 512],
                        start=True, stop=False)
                    nc.tensor.matmul(
                        p, CTn[1][:, tb * 128:(tb + 1) * 128],
                        wo_all[:, D + nck * 512:D + (nck + 1) * 512],
                        start=False, stop=True)
                    # tail units: ACT is idle after the last exp - split the
                    # psum drains across both engines
                    if wide and nck == 0:
                        nc.scalar.copy(ob[:, 0:512], p)
                    else:
                        nc.vector.tensor_copy(
                            ob[:, nck * 512:(nck + 1) * 512], p)
                nc.sync.dma_start(
                    out[tb * 128:(tb + 1) * 128, :], ob[:])

            # ---- fillers: ~one light unit per two key-block slots ---------
            # R0: V blocks 4-15 (0-3 emitted inline at round start).
            # R1: KT1/QT1 tck0-1 halves.  R2: KT1/QT1 tck2-3 halves.
            # R3: out-projection for tokens 0-1023.
            def halves(tcks):
                fl = []
                for w_all, b_all, dst in ((wk_all, bk_all, KT),
                                          (wq_all, bq_all, QT)):
                    for tck in tcks:
                        f = mk_qkhalf(w_all, b_all, dst, tck)
                        fl += [lambda f=f: f(0), lambda f=f: f(1)]
                return fl

            def spread(units, idx):
                fl = [None] * NT
                for i, u in enumerate(units):
                    fl[idx[i]] = u
                return fl

            fillers = {
                # V block tb at a slot before PV(tb) consumes it (slot tb+1)
                0: spread([(lambda tb=tb: v_unit(tb)) for tb in range(2, NT)],
                          list(range(1, 15))),
                # K halves first: R2-j12's QK needs KT1 tck3 by slot 12
                1: spread(halves((0, 1)), [1, 3, 5, 7, 9, 11, 13, 15]),
                2: spread(halves((2, 3)), [1, 3, 5, 7, 9, 11, 13, 15]),
                3: spread([(lambda tb=tb: out_unit(tb, psp, "sp", False))
                           for tb in range(8)], [1, 3, 5, 7, 9, 11, 13, 15]),
            }

            # ---- attention rounds -----------------------------------------
            # the previous round's normalize is emitted AFTER the next
            # round's first QK/ACT so the PE reaches them without stalling
            # the exp pipeline at round boundaries
            def normalize(fb, hc, ct, den):
                recS = rsp.tile([128, 512], F32, tag="rcs", name="recS")
                nc.vector.reciprocal_approx_fast(recS[:], den[:])
                recH = rsp.tile([128, 512], F16, tag="rch", name="recH")
                nc.vector.tensor_copy(recH[:], recS[:])
                for qc in range(2):
                    rep = psd.tile([128, 512], F32, tag="dn", name="rep")
                    r0, r1 = 32 * qc, 64 + 32 * qc
                    nc.tensor.matmul(
                        rep[0:64, :], onesR[r0:r0 + 1, :],
                        recH[r0:r0 + 1, 0:512],
                        start=True, stop=True, tile_position=(r0, 0))
                    nc.tensor.matmul(
                        rep[64:128, :], onesR[r1:r1 + 1, :],
                        recH[r1:r1 + 1, 0:512],
                        start=True, stop=True, tile_position=(r1, 64))
                    repS = rsp.tile([128, 512], F16, tag="rps", name="repS")
                    nc.vector.tensor_copy(repS[:], rep[:])
                    nc.vector.tensor_mul(
                        CTn[fb][:, hc + qc * 512:hc + (qc + 1) * 512],
                        ct[:, qc * 512:(qc + 1) * 512], repS[:])

            pending_norm = None
            pending_tail = None
            for rnd, (fb, half) in enumerate(((0, 0), (0, 1), (1, 0), (1, 1))):
                hc = half * 1024
                ct = den = None
                pts = {}
                fl = fillers[rnd]

                pvd = {}

                def mk_pv_den(ct, den, fb=fb, pts=pts):
                    pvd["f"] = lambda j: _pv_den(j, ct, den, fb, pts)

                def pv_den(j):
                    pvd["f"](j)

                def _pv_den(j, ct, den, fb, pts):
                    p0, p1 = pts[j]
                    first, last = (j == 0), (j == NT - 1)
                    for i, (pt, qc) in enumerate(((p0, 0), (p0, 1),
                                                  (p1, 0), (p1, 1))):
                        nc.tensor.matmul(
                            den[32 * i:32 * i + 1, :], onesP[:],
                            pt[:, qc * 512:(qc + 1) * 512],
                            start=first, stop=last,
                            tile_position=(0, 32 * i))
                    for qc in range(2):
                        qs = slice(qc * 512, (qc + 1) * 512)
                        nc.tensor.matmul(
                            ct[0:64, qs], V_t[j][:, fb * 128:fb * 128 + 64],
                            p0[:, qs], start=first, stop=last)
                        nc.tensor.matmul(
                            ct[64:128, qs],
                            V_t[j][:, fb * 128 + 64:fb * 128 + 128],
                            p1[:, qs], start=first, stop=last)

                for j in range(NT):
                    # ready work (prev PV/den, fillers) is emitted BEFORE this
                    # slot's QK so the PE never head-blocks on the st-buffer
                    # WAR while runnable instructions sit behind it
                    if j == 0 and pending_tail is not None:
                        pending_tail()
                        pending_tail = None
                    if j == 1:
                        if pending_norm is not None:
                            normalize(*pending_norm)
                            pending_norm = None
                        # ct/den allocated only now: their WAR deps (prior
                        # round's normalize reads) must already be emitted
                        ct = psc.tile([128, 1024], F32, tag="ct", name="ct")
                        den = psd.tile([128, 512], F32, tag="dn", name="den")
                        mk_pv_den(ct, den)
                    if j > 0:
                        pv_den(j - 1)
                    if rnd == 0 and j == 0:
                        for tb in range(2):
                            v_unit(tb)
                    if fl[j] is not None:
                        fl[j]()
                    st0 = pst.tile([128, 1024], F32, tag="st", name="st")
                    st1 = pst.tile([128, 1024], F32, tag="st", name="st")
                    for qc in range(2):
                        qs = slice(qc * 512, (qc + 1) * 512)
                        qsrc = slice(hc + qc * 512, hc + (qc + 1) * 512)
                        nc.tensor.matmul(
                            st0[:, qs], KT[fb][0:64, j * 128:(j + 1) * 128],
                            QT[fb][0:64, qsrc], start=True, stop=True)
                        nc.tensor.matmul(
                            st1[:, qs], KT[fb][64:128, j * 128:(j + 1) * 128],
                            QT[fb][64:128, qsrc], start=True, stop=True)
                    p0 = ptp.tile([128, 1024], F16, tag="pt", name="pt")
                    p1 = ptp.tile([128, 1024], F16, tag="pt", name="pt")
                    pts[j] = (p0, p1)
                    nc.scalar.activation(p0[:], st0[:], EXP, scale=0.125)
                    nc.scalar.activation(p1[:], st1[:], EXP, scale=0.125)
                if rnd < 3:
                    pending_tail = (lambda f=pvd["f"]: f(NT - 1))
                    pending_norm = (fb, hc, ct, den)
                else:
                    pv_den(NT - 1)
                    normalize(fb, hc, ct, den)

            # ---- remaining output blocks (tokens 1024:2048) ---------------
            for tb in range(8, NT):
                out_unit(tb, pst, "st", True)

    from concourse.library_overlay import lower_extended_insts
    lower_extended_insts(nc)   # populate .instr for InstCustomDveAnt (recip)
    _split_waits(nc)
    return nc


_NC = None


def _get_nc():
    global _NC
    if _NC is None:
        _NC = _build_program()
    return _NC


def _swz(a, dt=np.float16):
    """[C*128, F] -> [128, C*F]: chunk-major free layout, f16/f32 cast."""
    c = a.shape[0] // 128
    return np.ascontiguousarray(
        a.reshape(c, 128, -1).transpose(1, 0, 2).reshape(128, -1)).astype(dt)


def _shard_inputs(x, Wq, bq, Wk, bk, Wv, bv, Wo):
    xTs = [_swz(np.ascontiguousarray(x[b].T)) for b in range(2)]
    in_maps = []
    for core in range(8):
        b, g = divmod(core, 4)
        lo = g * GF
        in_maps.append({
            "xT": xTs[b],
            "Wq": _swz(Wq[:, lo:lo + GF]),
            "Wk": _swz(Wk[:, lo:lo + GF]),
            "Wv": _swz(Wv[:, lo:lo + GF]),
            "Wo": _swz(Wo[lo:lo + GF, :]),
            "bq": _swz(bq[lo:lo + GF].reshape(GF, 1), np.float32),
            "bk": _swz(bk[lo:lo + GF].reshape(GF, 1), np.float32),
        })
    return in_maps


def run(inputs, trace=False, trace_kwargs=None):
    """Run the kernel; returns (output [2,2048,1024] f32, BassKernelResults)."""
    inputs = {k: np.asarray(v, dtype=np.float32) for k, v in inputs.items()}
    in_maps = _shard_inputs(
        inputs["x"], inputs["Wq"], inputs["bq"], inputs["Wk"], inputs["bk"],
        inputs["Wv"], inputs["bv"], inputs["Wo"])
    nc = _get_nc()
    res = run_bass_kernel_spmd(
        nc, in_maps, list(range(8)), trace=trace, **(trace_kwargs or {}))
    # V bias commutes: softmax rows sum to 1, so ctx bias is bv exactly;
    # bo_eff = bo + bv @ Wo, added once per batch after the partial sum.
    bo_eff = inputs["bo"] + inputs["bv"] @ inputs["Wo"]
    out = np.empty((2, T, D), dtype=np.float32)
    for b in range(2):
        acc = None
        for g in range(4):
            part = res.results[4 * b + g]["out"].astype(np.float32)
            acc = part.copy() if acc is None else acc + part
        out[b] = acc + bo_eff[None, :]
    return out, res


def kernel(**inputs):
    out, _ = run(inputs, trace=False)
    return out


# revision 21
# speedup vs baseline: 1.1490x; 1.1490x over previous
"""Multi-head attention (b=2, t=2048, d=1024, h=16, hd=64) on 8 trn2 NeuronCores.

Sharding: core c = 4*b + g handles batch b and head-group g (4 heads =
2 fb-pairs, feature columns [g*256, (g+1)*256)). QKV column-sharded,
Wo row-sharded (Megatron); each core emits ONE pair-combined partial
[2048, 1024] f16 output; host sums the 4 group partials per batch and
adds bo_eff = bo + bv @ Wo (the V bias commutes through softmax:
softmax(S) @ (V + 1 bv^T) = softmax(S) @ V + 1 bv^T).

Schedule: 4 rounds (fb-pair, query-half) x 16 key blocks, paced by the
ACT engine (exp: 128 x [128,1024] instrs ~1.34us each, ~171us total);
PE work hides under it:
 - QK: two K=64 matmuls concurrent via row tiles (0,0)/(64,0).
 - PV: col tiles (0,0)/(0,64); head-even ctx to psum rows 0-63,
   head-odd to 64-127.
 - softmax denominators: four M=1 col-tiled matmuls (strips
   0/32/64/96) with a ones stationary vector, one psum bank.
 - 1/denominator: nc.vector.reciprocal_approx_fast.
 - V projected token-major (lhsT = xT chunk): no transposes.
Inputs arrive as 7 batched multi-dim-AP DMAs; the fb0 K/Q projections
run dc-outer, chasing the xT chunk DMAs, into borrowed attention psum
banks. PSUM (8 banks): st ping/pong 4 + ct 2 + den/rep 1 + scratch 1.
"""

import numpy as np

import concourse.bass as bass
import concourse.mybir as mybir
import concourse.tile as tile
from concourse.bass_utils import run_bass_kernel_spmd

F32 = mybir.dt.float32
F16 = mybir.dt.float16
EXP = mybir.ActivationFunctionType.Exp

T = 2048          # tokens per batch
D = 1024          # model dim
HD = 64           # head dim
GF = 256          # features per head-group (4 heads)
NT = T // 128     # 16 token/key blocks

MAX_WAITS = 1


def _split_waits(nc):
    """walrus in this container allows only one sync-wait per instruction;
    hoist extras onto same-engine NoOps immediately before the offender."""
    for f in nc.m.functions:
        for blk in f.blocks:
            insts = list(blk.instructions)
            new, changed = [], False
            for ins in insts:
                si = ins.sync_info
                waits = list(si.on_wait) if si and si.on_wait else []
                if len(waits) > MAX_WAITS:
                    changed = True
                    extra, keep = waits[:-MAX_WAITS], waits[-MAX_WAITS:]
                    for i in range(0, len(extra), MAX_WAITS):
                        new.append(mybir.InstNoOp(
                            name=f"{ins.name}-wsplit{i}",
                            engine=ins.engine,
                            sync_info=mybir.SyncInfo(
                                on_wait=extra[i:i + MAX_WAITS], on_update=[]),
                        ))
                    ins.sync_info = mybir.SyncInfo(
                        on_wait=keep,
                        on_update=list(si.on_update) if si.on_update else [])
                new.append(ins)
            if changed:
                blk.instructions = new


def _build_program():
    nc = bass.Bass("TRN2", target_bir_lowering=False, debug=False, num_devices=8)

    # all inputs host-swizzled to [128, free] SBUF layout, chunk-major
    xT = nc.dram_tensor("xT", [128, 8 * T], F16, kind="ExternalInput")
    Wq = nc.dram_tensor("Wq", [128, 8 * GF], F16, kind="ExternalInput")
    Wk = nc.dram_tensor("Wk", [128, 8 * GF], F16, kind="ExternalInput")
    Wv = nc.dram_tensor("Wv", [128, 8 * GF], F16, kind="ExternalInput")
    Wo = nc.dram_tensor("Wo", [128, 2 * D], F16, kind="ExternalInput")
    bq = nc.dram_tensor("bq", [128, 2], F32, kind="ExternalInput")
    bk = nc.dram_tensor("bk", [128, 2], F32, kind="ExternalInput")
    out = nc.dram_tensor("out", [T, D], F16, kind="ExternalOutput")

    with tile.TileContext(nc) as tc:
        with (
            nc.allow_low_precision(reason="fp16 rounding is intentional"),
            tc.tile_pool(name="w", bufs=1) as wp,        # persistent tiles
            tc.tile_pool(name="pt", bufs=6) as ptp,      # probs tiles
            tc.tile_pool(name="rs", bufs=2) as rsp,      # recip staging
            tc.tile_pool(name="ob", bufs=3) as obp,      # out staging
            tc.tile_pool(name="st", bufs=2, space="PSUM") as pst,   # scores
            tc.tile_pool(name="ct", bufs=1, space="PSUM") as psc,   # ctx accum
            tc.tile_pool(name="dn", bufs=1, space="PSUM") as psd,   # denom/rep
            tc.tile_pool(name="sp", bufs=1, space="PSUM") as psp,   # scratch
        ):
            # ---- ACT table preload: dummy exp at t0 (hides the ~2.7us
            # PSEUDO_LOAD_ACT_FUNC_SET under the input DMAs) ----------------
            warm_i = wp.tile([1, 16], F32, tag="warm_i")
            nc.gpsimd.memset(warm_i[:], 0.0)
            warm_o = wp.tile([1, 16], F16, tag="warm_o")
            nc.scalar.activation(warm_o[:], warm_i[:], EXP, scale=1.0)

            # ---- batched input DMAs. All tensors are pre-swizzled on the
            # host to the exact [128, free] SBUF layout (chunk-major along
            # free), so every DMA is a contiguous per-partition run --------
            wk_all = wp.tile([128, 8 * GF], F16, tag="wk")
            nc.sync.dma_start(wk_all[:], Wk[:])
            wq_all = wp.tile([128, 8 * GF], F16, tag="wq")
            nc.sync.dma_start(wq_all[:], Wq[:])
            xTt = wp.tile([128, 8 * T], F16, tag="xt")
            for lo, hi in ((0, 1), (1, 2), (2, 4), (4, 6), (6, 8)):
                nc.sync.dma_start(
                    xTt[:, lo * T:hi * T], xT[:, lo * T:hi * T])
            wv_all = wp.tile([128, 8 * GF], F16, tag="wv")
            nc.sync.dma_start(wv_all[:], Wv[:])
            bk_all = wp.tile([128, 2], F32, tag="bk")
            nc.sync.dma_start(bk_all[:], bk[:])
            bq_all = wp.tile([128, 2], F32, tag="bq")
            nc.sync.dma_start(bq_all[:], bq[:])
            wo_all = wp.tile([128, 2 * D], F16, tag="wo")
            nc.sync.dma_start(wo_all[:], Wo[:])

            def xc(dc, lo, hi):          # xT chunk dc, token slice
                return xTt[:, dc * T + lo:dc * T + hi]

            def wsl(w, dc, lo, hi):      # weight chunk dc, feature slice
                return w[:, dc * GF + lo:dc * GF + hi]

            onesP = wp.tile([128, 1], F16, tag="onesP")   # denom lhsT
            nc.gpsimd.memset(onesP[:], 1.0)
            onesR = wp.tile([128, 64], F16, tag="onesR")  # recip-replicate lhsT
            nc.gpsimd.memset(onesR[:], 1.0)
            CTn0g = wp.tile([128, 512], F16, tag="wrg")   # warmup rhs (garbage)
            nc.gpsimd.memset(CTn0g[:], 0.0)

            # PE warmup: dummy matmuls (no DMA deps) spanning until the xT
            # chunks land, so the HAM clock-gate is 8/8 for the chains
            wrm = pst.tile([128, 1024], F32, tag="st", name="wrm")
            for i in range(32):
                nc.tensor.matmul(wrm[0:64, 0:512], onesR[:, 0:64],
                                 CTn0g[:, 0:512], start=True, stop=True)

            # Q^T/K^T feature-major: rows 0-63 head-even(2fb), 64-127 head-odd
            QT = [wp.tile([128, T], F16, tag=f"qt{fb}", name=f"qt{fb}")
                  for fb in range(2)]
            KT = [wp.tile([128, T], F16, tag=f"kt{fb}", name=f"kt{fb}")
                  for fb in range(2)]
            # V token-major: [128 tokens, 4 heads x 64]
            V_t = [wp.tile([128, GF], F16, tag=f"v{tb}", name=f"v{tb}")
                   for tb in range(NT)]
            # normalized ctx, pair-feature-major: [128 feats, T]
            CTn = [wp.tile([128, T], F16, tag=f"ctn{fb}", name=f"ctn{fb}")
                   for fb in range(2)]

            # ---- pre-phase: fb0 K/Q projections, dc-outer, chasing the xT
            # chunk DMAs; psum = the 4 attention banks (idle until round 0)
            preK = [pst.tile([128, 1024], F32, tag="st", name=f"preK{i}")
                    for i in range(2)]
            preQ01 = psc.tile([128, 1024], F32, tag="ct", name="preQ01")
            preQ2 = psd.tile([128, 512], F32, tag="dn", name="preQ2")
            preQ3 = psp.tile([128, 512], F32, tag="sp", name="preQ3")
            def pre_mm(kind, tck, dc):
                first, last = dc == 0, dc == 7
                if kind == 0:
                    dst = preK[tck // 2][:,
                                         (tck % 2) * 512:(tck % 2) * 512 + 512]
                    w = wk_all
                else:
                    dst = (preQ01[:, 0:512], preQ01[:, 512:1024],
                           preQ2[:], preQ3[:])[tck]
                    w = wq_all
                nc.tensor.matmul(
                    dst, wsl(w, dc, 0, 128),
                    xc(dc, tck * 512, (tck + 1) * 512),
                    start=first, stop=last)

            for dc in range(7):
                for tck in range(4):
                    pre_mm(0, tck, dc)
                for tck in range(4):
                    pre_mm(1, tck, dc)
            # dc7 in j0-gating order: KT tck0, QT tck0/1 complete first
            for kind, tck in ((0, 0), (1, 0), (1, 1), (0, 1),
                              (0, 2), (0, 3), (1, 2), (1, 3)):
                pre_mm(kind, tck, 7)
            # bias casts, in first-use order (QK j0 needs KT tck0 + QT tck0/1)
            for (dst, src, b_all) in (
                    (KT[0][:, 0:512], preK[0][:, 0:512], bk_all),
                    (QT[0][:, 0:512], preQ01[:, 0:512], bq_all),
                    (QT[0][:, 512:1024], preQ01[:, 512:1024], bq_all),
                    (KT[0][:, 512:1024], preK[0][:, 512:1024], bk_all),
                    (KT[0][:, 1024:1536], preK[1][:, 0:512], bk_all),
                    (KT[0][:, 1536:2048], preK[1][:, 512:1024], bk_all),
                    (QT[0][:, 1024:1536], preQ2[:], bq_all),
                    (QT[0][:, 1536:2048], preQ3[:], bq_all)):
                nc.vector.tensor_scalar_add(dst, src, b_all[:, 0:1])

            # Q/K projection filler halves (fb1), psum via the scratch bank
            def qk_half(w_all, b_all, dst, tck, phase, sp):
                for dc in range(4 * phase, 4 * phase + 4):
                    nc.tensor.matmul(
                        sp[:, 0:512], wsl(w_all, dc, 128, 256),
                        xc(dc, tck * 512, (tck + 1) * 512),
                        start=(dc == 0), stop=(dc == 7))
                if phase == 1:
                    nc.vector.tensor_scalar_add(
                        dst[1][:, tck * 512:(tck + 1) * 512], sp[:, 0:512],
                        b_all[:, 1:2])

            def mk_qkhalf(w_all, b_all, dst, tck):
                st = {}

                def f(phase):
                    if phase == 0:
                        st["sp"] = psp.tile([128, 512], F32, tag="sp",
                                            name="fill_sp")
                    qk_half(w_all, b_all, dst, tck, phase, st["sp"])
                return f

            # V projection, token-major (no bias - folded into host bo_eff)
            def v_unit(tb):
                vp = psp.tile([128, GF], F32, tag="sp", name="vp")
                for dc in range(8):
                    nc.tensor.matmul(
                        vp[:], xc(dc, tb * 128, (tb + 1) * 128),
                        wsl(wv_all, dc, 0, 256),
                        start=(dc == 0), stop=(dc == 7))
                nc.vector.tensor_copy(V_t[tb][:], vp[:])

            # pair-combined output projection for token block tb
            def out_unit(tb, pool, tag, wide):
                ob = obp.tile([128, D], F16, tag="o", name="o")
                spw = pool.tile([128, 1024], F32, tag=tag, name="osp") \
                    if wide else None
                for nck in range(2):
                    if wide:
                        p = spw[:, nck * 512:(nck + 1) * 512]
                    else:
                        sp = pool.tile([128, 512], F32, tag=tag, name="osp")
                        p = sp[:, 0:512]
                    nc.tensor.matmul(
                        p, CTn[0][:, tb * 128:(tb + 1) * 128],
                        wo_all[:, nck * 512:(nck + 1) * 512],
                        start=True, stop=False)
                    nc.tensor.matmul(
                        p, CTn[1][:, tb * 128:(tb + 1) * 128],
                        wo_all[:, D + nck * 512:D + (nck + 1) * 512],
                        start=False, stop=True)
                    # tail units: ACT is idle after the last exp - split the
                    # psum drains across both engines
                    if wide and nck == 0:
                        nc.scalar.copy(ob[:, 0:512], p)
                    else:
                        nc.vector.tensor_copy(
                            ob[:, nck * 512:(nck + 1) * 512], p)
                nc.sync.dma_start(
                    out[tb * 128:(tb + 1) * 128, :], ob[:])

            # ---- fillers: ~one light unit per two key-block slots ---------
            # R0: V blocks 4-15 (0-3 emitted inline at round start).
            # R1: KT1/QT1 tck0-1 halves.  R2: KT1/QT1 tck2-3 halves.
            # R3: out-projection for tokens 0-1023.
            def halves(tcks):
                fl = []
                for w_all, b_all, dst in ((wk_all, bk_all, KT),
                                          (wq_all, bq_all, QT)):
                    for tck in tcks:
                        f = mk_qkhalf(w_all, b_all, dst, tck)
                        fl += [lambda f=f: f(0), lambda f=f: f(1)]
                return fl

            def spread(units, idx):
                fl = [None] * NT
                for i, u in enumerate(units):
                    fl[idx[i]] = u
                return fl

            fillers = {
                # V block tb at a slot before PV(tb) consumes it (slot tb+1)
                0: spread([(lambda tb=tb: v_unit(tb)) for tb in range(2, NT)],
                          list(range(1, 15))),
                # K halves first: R2-j12's QK needs KT1 tck3 by slot 12
                1: spread(halves((0, 1)), [1, 3, 5, 7, 9, 11, 13, 15]),
                2: spread(halves((2, 3)), [1, 3, 5, 7, 9, 11, 13, 15]),
                3: spread([(lambda tb=tb: out_unit(tb, psp, "sp", False))
                           for tb in range(8)], [1, 3, 5, 7, 9, 11, 13, 15]),
            }

            # ---- attention rounds -----------------------------------------
            # the previous round's normalize is emitted AFTER the next
            # round's first QK/ACT so the PE reaches them without stalling
            # the exp pipeline at round boundaries
            def normalize(fb, hc, ct, den):
                recS = rsp.tile([128, 512], F32, tag="rcs", name="recS")
                nc.vector.reciprocal_approx_fast(recS[:], den[:])
                recH = rsp.tile([128, 512], F16, tag="rch", name="recH")
                nc.vector.tensor_copy(recH[:], recS[:])
                for qc in range(2):
                    rep = psd.tile([128, 512], F32, tag="dn", name="rep")
                    r0, r1 = 32 * qc, 64 + 32 * qc
                    nc.tensor.matmul(
                        rep[0:64, :], onesR[r0:r0 + 1, :],
                        recH[r0:r0 + 1, 0:512],
                        start=True, stop=True, tile_position=(r0, 0))
                    nc.tensor.matmul(
                        rep[64:128, :], onesR[r1:r1 + 1, :],
                        recH[r1:r1 + 1, 0:512],
                        start=True, stop=True, tile_position=(r1, 64))
                    repS = rsp.tile([128, 512], F16, tag="rps", name="repS")
                    nc.vector.tensor_copy(repS[:], rep[:])
                    nc.vector.tensor_mul(
                        CTn[fb][:, hc + qc * 512:hc + (qc + 1) * 512],
                        ct[:, qc * 512:(qc + 1) * 512], repS[:])

            pending_norm = None
            pending_tail = None
            for rnd, (fb, half) in enumerate(((0, 0), (0, 1), (1, 0), (1, 1))):
                hc = half * 1024
                ct = den = None
                pts = {}
                fl = fillers[rnd]

                pvd = {}

                def mk_pv_den(ct, den, fb=fb, pts=pts):
                    pvd["f"] = lambda j: _pv_den(j, ct, den, fb, pts)

                def pv_den(j):
                    pvd["f"](j)

                def _pv_den(j, ct, den, fb, pts):
                    p0, p1 = pts[j]
                    first, last = (j == 0), (j == NT - 1)
                    for i, (pt, qc) in enumerate(((p0, 0), (p0, 1),
                                                  (p1, 0), (p1, 1))):
                        nc.tensor.matmul(
                            den[32 * i:32 * i + 1, :], onesP[:],
                            pt[:, qc * 512:(qc + 1) * 512],
                            start=first, stop=last,
                            tile_position=(0, 32 * i))
                    for qc in range(2):
                        qs = slice(qc * 512, (qc + 1) * 512)
                        nc.tensor.matmul(
                            ct[0:64, qs], V_t[j][:, fb * 128:fb * 128 + 64],
                            p0[:, qs], start=first, stop=last)
                        nc.tensor.matmul(
                            ct[64:128, qs],
                            V_t[j][:, fb * 128 + 64:fb * 128 + 128],
                            p1[:, qs], start=first, stop=last)

                for j in range(NT):
                    st0 = pst.tile([128, 1024], F32, tag="st", name="st")
                    st1 = pst.tile([128, 1024], F32, tag="st", name="st")
                    if j == 1:
                        if pending_norm is not None:
                            normalize(*pending_norm)
                            pending_norm = None
                        # ct/den allocated only now: their WAR deps (prior
                        # round's normalize reads) must already be emitted
                        ct = psc.tile([128, 1024], F32, tag="ct", name="ct")
                        den = psd.tile([128, 512], F32, tag="dn", name="den")
                        mk_pv_den(ct, den)
                    for qc in range(2):
                        qs = slice(qc * 512, (qc + 1) * 512)
                        qsrc = slice(hc + qc * 512, hc + (qc + 1) * 512)
                        nc.tensor.matmul(
                            st0[:, qs], KT[fb][0:64, j * 128:(j + 1) * 128],
                            QT[fb][0:64, qsrc], start=True, stop=True)
                        nc.tensor.matmul(
                            st1[:, qs], KT[fb][64:128, j * 128:(j + 1) * 128],
                            QT[fb][64:128, qsrc], start=True, stop=True)
                    p0 = ptp.tile([128, 1024], F16, tag="pt", name="pt")
                    p1 = ptp.tile([128, 1024], F16, tag="pt", name="pt")
                    pts[j] = (p0, p1)
                    nc.scalar.activation(p0[:], st0[:], EXP, scale=0.125)
                    nc.scalar.activation(p1[:], st1[:], EXP, scale=0.125)
                    if j == 0 and pending_tail is not None:
                        pending_tail()
                        pending_tail = None
                    if rnd == 0 and j == 0:
                        for tb in range(2):
                            v_unit(tb)
                    if j > 0:
                        pv_den(j - 1)
                    if fl[j] is not None:
                        fl[j]()
                if rnd < 3:
                    pending_tail = (lambda f=pvd["f"]: f(NT - 1))
                    pending_norm = (fb, hc, ct, den)
                else:
                    pv_den(NT - 1)
                    normalize(fb, hc, ct, den)

            # ---- remaining output blocks (tokens 1024:2048) ---------------
            for tb in range(8, NT):
                out_unit(tb, pst, "st", True)

    from concourse.library_overlay import lower_extended_insts
    lower_extended_insts(nc)   # populate .instr for InstCustomDveAnt (recip)
    _split_waits(nc)
    return nc


_NC = None


def _get_nc():
    global _NC
    if _NC is None:
        _NC = _build_program()
    return _NC


def _swz(a, dt=np.float16):
    """[C*128, F] -> [128, C*F]: chunk-major free layout, f16/f32 cast."""
    c = a.shape[0] // 128
    return np.ascontiguousarray(
        a.reshape(c, 128, -1).transpose(1, 0, 2).reshape(128, -1)).astype(dt)


def _shard_inputs(x, Wq, bq, Wk, bk, Wv, bv, Wo):
    xTs = [_swz(np.ascontiguousarray(x[b].T)) for b in range(2)]
    in_maps = []
    for core in range(8):
        b, g = divmod(core, 4)
        lo = g * GF
        in_maps.append({
            "xT": xTs[b],
            "Wq": _swz(Wq[:, lo:lo + GF]),
            "Wk": _swz(Wk[:, lo:lo + GF]),
            "Wv": _swz(Wv[:, lo:lo + GF]),
            "Wo": _swz(Wo[lo:lo + GF, :]),
            "bq": _swz(bq[lo:lo + GF].reshape(GF, 1), np.float32),
            "bk": _swz(bk[lo:lo + GF].reshape(GF, 1), np.float32),
        })
    return in_maps


def run(inputs, trace=False, trace_kwargs=None):
    """Run the kernel; returns (output [2,2048,1024] f32, BassKernelResults)."""
    inputs = {k: np.asarray(v, dtype=np.float32) for k, v in inputs.items()}
    in_maps = _shard_inputs(
        inputs["x"], inputs["Wq"], inputs["bq"], inputs["Wk"], inputs["bk"],
        inputs["Wv"], inputs["bv"], inputs["Wo"])
    nc = _get_nc()
    res = run_bass_kernel_spmd(
        nc, in_maps, list(range(8)), trace=trace, **(trace_kwargs or {}))
    # V bias commutes: softmax rows sum to 1, so ctx bias is bv exactly;
    # bo_eff = bo + bv @ Wo, added once per batch after the partial sum.
    bo_eff = inputs["bo"] + inputs["bv"] @ inputs["Wo"]
    out = np.empty((2, T, D), dtype=np.float32)
    for b in range(2):
        acc = None
        for g in range(4):
            part = res.results[4 * b + g]["out"].astype(np.float32)
            acc = part.copy() if acc is None else acc + part
        out[b] = acc + bo_eff[None, :]
    return out, res


def kernel(**inputs):
    out, _ = run(inputs, trace=False)
    return out
